# revision 33
# baseline (speedup 1.0000x reference)
"""Trainium2 Bass kernel for dynamic-depthwise + static conv module.

Computation (per batch b, channel c):
  hid  = leaky_relu(k_v @ W1.T, 0.1)
  kern = (hid @ W2.T).reshape(b*c, 3, 3)        # per-(b,c) dynamic 3x3
  dyn  = leaky_relu(depthwise3x3(x, kern), 0.1)
  res  = conv3x3(x, conv_w) + conv_b
  out  = dyn + res

Sharding: pure data-parallel, B=16 over 8 cores (2 batches/core).

Per-core layout:
  x_pad [128 part = 2 batches x 64 ch, 194*194 zero-padded bf16].  The
  padding is baked into the DRAM tensor host-side, so the load is a
  plain contiguous 128-partition bf16 DMA (no casts, no strided
  descriptors, no on-chip memsets except the stream guards).
  For each spatial tile (2 padded rows, N=388) and each of 9 taps:
  one bf16 matmul per batch, lhsT[k=ci, m] = [conv_w tap | diag(kern)]
  -> PSUM bank per batch: parts 0-63 = static conv, parts 64-127 =
  dynamic.  The two batches run concurrently via PE row tiling (rows
  0-63 / 64-127), which sustains ~2 rows/cycle aggregate.  ACT evicts
  the dynamic half with Prelu (f32); DVE adds bias to the static half;
  a f32 SWDGE accumulate-DMA (dst partitions 0-63 only; larger RMW
  runs than 6 KiB/partition miscompute on HW) folds them; DMA to HBM.
"""
import numpy as np

import concourse.bass as bass
import concourse.tile as tile
import concourse.mybir as mybir

F32 = mybir.dt.float32
F32R = mybir.dt.float32r
BF16 = mybir.dt.bfloat16

B, C, H, W = 16, 64, 192, 192
NCORES = 8
BLOC = B // NCORES          # batches per core
WP = W + 2                  # padded row width
HP = H + 2
PADQ = WP * HP
G = 4                       # guard elems each side of padded buffer
HW = H * W
NTILE = 388                 # 2 padded rows per tile
NT = H // 2                 # 96 tiles

TAPS = [(dy, dx) for dy in (-1, 0, 1) for dx in (-1, 0, 1)]


def _legalize_waits(nc, max_waits=1, evsem_waits=2):
    """This walrus build rejects >1 sync wait on most instructions (2 on
    EventSemaphore). Spill excess waits onto same-engine EventSemaphores
    placed immediately before the instruction."""
    for f in nc.m.functions:
        for bb in f.blocks:
            new_insts = []
            for inst in bb.instructions:
                si = inst.sync_info
                if si is not None and si.on_wait and len(si.on_wait) > max_waits:
                    waits = list(si.on_wait)
                    keep = waits[-max_waits:]
                    spill = waits[:-max_waits]
                    while spill:
                        chunk, spill = spill[:evsem_waits], spill[evsem_waits:]
                        ev = mybir.InstEventSemaphore(
                            name=nc.get_next_instruction_name(),
                            engine=inst.engine,
                            ins=[],
                            outs=[],
                            sync_info=mybir.SyncInfo(on_wait=chunk, on_update=[]),
                        )
                        nc.register_instruction(ev)
                        new_insts.append(ev)
                    inst.sync_info = mybir.SyncInfo(
                        on_wait=keep, on_update=list(si.on_update or [])
                    )
                new_insts.append(inst)
            bb.instructions[:] = new_insts


def _build_nc(loopk=1, xrep=False, hwloop=False):
    nc = bass.Bass()
    xs = nc.dram_tensor("xs", (128, PADQ), BF16, kind="ExternalInput")
    mlpw = nc.dram_tensor("mlpw", (64, BLOC + 64 + 576), F32R,
                          kind="ExternalInput")
    wstat = nc.dram_tensor("wstat", (128, 9 * 128), BF16,
                           kind="ExternalInput")
    ident = nc.dram_tensor("ident", (128, 64), BF16, kind="ExternalInput")
    biasd = nc.dram_tensor("biasd", (128, 1), F32, kind="ExternalInput")
    out = nc.dram_tensor("out", (128, HW), F32, kind="ExternalOutput")

    with tile.TileContext(nc) as tc:
        with (
            tc.tile_pool(name="big", bufs=1) as big,
            tc.tile_pool(name="wpool", bufs=1) as wpool,
            tc.tile_pool(name="work", bufs=3) as work,
        ):
            # ---- persistent tiles ----
            x_pad = big.tile([128, G + PADQ + G], BF16, tag="x_pad")
            wbuf = wpool.tile([128, 9 * 128], BF16, tag="wbuf")
            id_t = wpool.tile([128, 64], BF16, tag="id_t")
            bias_t = wpool.tile([128, 1], F32, tag="bias_t")
            mlpw_t = wpool.tile([64, BLOC + 64 + 576], F32R, tag="mlpw_t")
            kvT_t = mlpw_t[:, 0:BLOC]
            w1t_t = mlpw_t[:, BLOC:BLOC + 64]
            w2t_t = mlpw_t[:, BLOC + 64:BLOC + 64 + 576]
            kern_flat = wpool.tile([BLOC, 576], F32, tag="kern_flat")
            kern128 = wpool.tile([128, 9], F32, tag="kern128")
            hidT = wpool.tile([64, BLOC], F32R, tag="hidT")

            # pre-warm the ACT Prelu table while DMAs run (no input deps)
            warm = wpool.tile([1, 1], F32, tag="warm")
            nc.vector.memset(warm[:], 0.0)
            nc.scalar.activation(warm[:], warm[:],
                                 mybir.ActivationFunctionType.Prelu, alpha=0.1)

            # ---- small constant / weight loads (SP queue, idle here) ----
            nc.sync.dma_start(mlpw_t[:], mlpw[:])
            nc.sync.dma_start(wbuf[:], wstat[:])
            nc.sync.dma_start(id_t[:], ident[:])
            nc.sync.dma_start(bias_t[:], biasd[:])

            # guard zeros (padding itself is baked into the DRAM layout)
            xbv = x_pad.bitcast(mybir.dt.uint16)
            nc.vector.memset(xbv[:, 0:G], 0)
            nc.vector.memset(xbv[:, G + PADQ:G + PADQ + G], 0)

            # ---- MLP: kern = (lrelu(k_v @ W1.T) @ W2.T), f32r ----
            # Scoped PSUM pool: released before pmain so the main loop can
            # use all 8 banks.
            with tc.tile_pool(name="pmlp", bufs=2, space="PSUM") as pmlp:
                p_hid = pmlp.tile([64, 512], F32, tag="pmlp")
                nc.tensor.matmul(p_hid[0:64, 0:BLOC], w1t_t, kvT_t,
                                 start=True, stop=True)
                nc.scalar.activation(hidT[:], p_hid[0:64, 0:BLOC],
                                     mybir.ActivationFunctionType.Prelu,
                                     alpha=0.1)
                p_k1 = pmlp.tile([64, 512], F32, tag="pmlp")
                p_k2 = pmlp.tile([64, 512], F32, tag="pmlp")
                nc.tensor.matmul(p_k1[0:BLOC, 0:288], hidT[:], w2t_t[:, 0:288],
                                 start=True, stop=True)
                nc.tensor.matmul(p_k2[0:BLOC, 0:288], hidT[:],
                                 w2t_t[:, 288:576], start=True, stop=True)
                nc.scalar.copy(kern_flat[:, 0:288], p_k1[0:BLOC, 0:288])
                nc.scalar.copy(kern_flat[:, 288:576], p_k2[0:BLOC, 0:288])
            # reshape (BLOC, 576) -> (128, 9): partition bc = b*64+c
            for b in range(BLOC):
                nc.sync.dma_start(kern128[b * 64:(b + 1) * 64, :],
                                  kern_flat[b:b + 1, :])
            # fill diagonal blocks of wbuf: cols t*128+64 .. t*128+128
            for t in range(9):
                nc.vector.tensor_scalar(
                    wbuf[:, t * 128 + 64:t * 128 + 128], id_t[:],
                    kern128[:, t:t + 1], None, op0=mybir.AluOpType.mult)

            # ---- x loads: contiguous 128-partition bf16 chunks ----
            NCHUNK = 8
            ROWS_PER_CHUNK = (HP + NCHUNK - 1) // NCHUNK   # 25 padded rows

            def load_x_chunk(c):
                if c >= NCHUNK:
                    if not xrep:
                        return
                    c = c % NCHUNK
                r0 = c * ROWS_PER_CHUNK
                r1 = min(HP, r0 + ROWS_PER_CHUNK)
                nc.scalar.dma_start(x_pad[:, G + r0 * WP:G + r1 * WP],
                                    xs[:, r0 * WP:r1 * WP])

            # ---- main loop over 96 spatial tiles (2 padded rows each),
            # grouped by 4 for batched accumulate-DMA + output DMA ----
            GT = 4
            NG = NT // GT
            PREFETCH = 3
            pmain_ctx = tc.tile_pool(name="pmain", bufs=4, space="PSUM")
            pmain = pmain_ctx.__enter__()

            def emit_group(g, do_load, rep=0):
                # 24 groups consume 8 chunks; issue one every 3rd group
                if do_load and g % 3 == 0:
                    load_x_chunk(rep * NCHUNK + g // 3 + PREFETCH)
                dst0 = work.tile([128, GT, 2, W], BF16, tag="dst0", bufs=3)
                dst1 = work.tile([128, GT, 2, W], BF16, tag="dst1", bufs=3)
                st0 = work.tile([64, GT, 2, W], F32, tag="st0", bufs=3)
                st1 = work.tile([64, GT, 2, W], F32, tag="st1", bufs=3)
                for gi in range(GT):
                    ti = g * GT + gi
                    hp0 = 1 + 2 * ti
                    qs = hp0 * WP
                    pb0f = pmain.tile([128, 512], F32, tag="pb0")
                    pb1f = pmain.tile([128, 512], F32, tag="pb1")
                    pb0 = pb0f[:, 0:NTILE]
                    pb1 = pb1f[:, 0:NTILE]
                    for t, (dy, dx) in enumerate(TAPS):
                        base = G + qs + dy * WP + dx
                        nc.tensor.matmul(
                            pb0[:], wbuf[0:64, t * 128:(t + 1) * 128],
                            x_pad[0:64, base:base + NTILE],
                            start=(t == 0), stop=(t == 8), tile_position=(0, 0))
                        nc.tensor.matmul(
                            pb1[:], wbuf[64:128, t * 128:(t + 1) * 128],
                            x_pad[64:128, base:base + NTILE],
                            start=(t == 0), stop=(t == 8), tile_position=(64, 0))
                    # dyn halves: Prelu -> compact staging (parts 64-127)
                    pv0d = pb0[64:128, :].rearrange("p (r w) -> p r w", w=WP)[:, :, 1:1 + W]
                    pv1d = pb1[64:128, :].rearrange("p (r w) -> p r w", w=WP)[:, :, 1:1 + W]
                    nc.scalar.activation(dst0[64:128, gi, :, :], pv0d,
                                         mybir.ActivationFunctionType.Prelu,
                                         alpha=0.1)
                    nc.scalar.activation(dst1[64:128, gi, :, :], pv1d,
                                         mybir.ActivationFunctionType.Prelu,
                                         alpha=0.1)
                    # static halves: add bias -> compact staging (parts 0-63)
                    pv0 = pb0[0:64, :].rearrange("p (r w) -> p r w", w=WP)[:, :, 1:1 + W]
                    pv1 = pb1[0:64, :].rearrange("p (r w) -> p r w", w=WP)[:, :, 1:1 + W]
                    nc.vector.tensor_scalar(st0[:, gi, :, :], pv0,
                                            bias_t[0:64, :], None,
                                            op0=mybir.AluOpType.add)
                    nc.vector.tensor_scalar(st1[:, gi, :, :], pv1,
                                            bias_t[0:64, :], None,
                                            op0=mybir.AluOpType.add)
                # cross-partition add of leaky(dyn) via SWDGE accumulate DMA
                h0 = 2 * GT * g
                if g < NG - 1:
                    nc.gpsimd.dma_start(st0[:], dst0[64:128, :, :, :],
                                        accum_op=mybir.AluOpType.add)
                    nc.gpsimd.dma_start(st1[:], dst1[64:128, :, :, :],
                                        accum_op=mybir.AluOpType.add)
                    nc.sync.dma_start(out[0:64, h0 * W:(h0 + 2 * GT) * W], st0[:])
                    nc.sync.dma_start(out[64:128, h0 * W:(h0 + 2 * GT) * W],
                                      st1[:])
                else:
                    # finer-grained tail: per-tile accum + out DMAs
                    for gi in range(GT):
                        hh = h0 + 2 * gi
                        nc.gpsimd.dma_start(st0[:, gi, :, :],
                                            dst0[64:128, gi, :, :],
                                            accum_op=mybir.AluOpType.add)
                        nc.gpsimd.dma_start(st1[:, gi, :, :],
                                            dst1[64:128, gi, :, :],
                                            accum_op=mybir.AluOpType.add)
                        nc.sync.dma_start(out[0:64, hh * W:(hh + 2) * W],
                                          st0[:, gi, :, :])
                        nc.sync.dma_start(out[64:128, hh * W:(hh + 2) * W],
                                          st1[:, gi, :, :])

            for c in range(PREFETCH):
                load_x_chunk(c)
            for rep in range(loopk):
                for g in range(NG):
                    emit_group(g, do_load=True, rep=rep)
            pmain_ctx.__exit__(None, None, None)

    _legalize_waits(nc)
    return nc


_NC_CACHE = {}


def _get_nc(loopk=1, xrep=False, hwloop=False):
    key = (loopk, xrep, hwloop)
    if key not in _NC_CACHE:
        _NC_CACHE[key] = _build_nc(loopk, xrep, hwloop)
    return _NC_CACHE[key]


def make_in_maps(x, k_v, W1, W2, conv_w, conv_b):
    import ml_dtypes

    # host-side weight layout prep (parameters only; no input-dependent math)
    blocks = []
    for dy in (0, 1, 2):
        for dx in (0, 1, 2):
            lhsT = conv_w[:, :, dy, dx].T          # [ci, co]
            blocks.append(np.concatenate(
                [lhsT, np.zeros((64, 64), np.float32)], axis=1))
    wstat_half = np.concatenate(blocks, axis=1)     # [64, 9*128]
    wstat = np.tile(wstat_half, (2, 1)).astype(ml_dtypes.bfloat16)
    ident = np.tile(np.eye(64, dtype=np.float32), (2, 1)).astype(
        ml_dtypes.bfloat16)
    biasd = np.tile(conv_b, 2)[:, None].astype(np.float32)
    w1t = W1.T.copy()                               # [64, 64]
    w2t = W2.T.copy()                               # [64, 576]

    # x: zero-padded bf16, baked host-side; partition bc = b*64 + c
    xb = x.astype(ml_dtypes.bfloat16)               # (B, C, H, W)
    in_maps = []
    for cidx in range(NCORES):
        xpad = np.zeros((128, HP, WP), ml_dtypes.bfloat16)
        xpad[:, 1:1 + H, 1:1 + W] = xb[cidx * BLOC:(cidx + 1) * BLOC].reshape(
            128, H, W)
        kvT = k_v[cidx * BLOC:(cidx + 1) * BLOC].T.copy()  # [64, BLOC]
        mlpw = np.concatenate([kvT, w1t, w2t], axis=1)
        in_maps.append({
            "xs": xpad.reshape(128, PADQ), "mlpw": mlpw,
            "wstat": wstat, "ident": ident, "biasd": biasd,
        })
    return in_maps


def expected_core0(full):
    return full[0:BLOC].reshape(128, HW)


def kernel(x, k_v, W1, W2, conv_w, conv_b):
    from concourse.bass_utils import run_bass_kernel_spmd

    x = np.ascontiguousarray(x, dtype=np.float32)
    k_v = np.ascontiguousarray(k_v, dtype=np.float32)
    W1 = np.ascontiguousarray(W1, dtype=np.float32)
    W2 = np.ascontiguousarray(W2, dtype=np.float32)
    conv_w = np.ascontiguousarray(conv_w, dtype=np.float32)
    conv_b = np.ascontiguousarray(conv_b, dtype=np.float32)

    in_maps = make_in_maps(x, k_v, W1, W2, conv_w, conv_b)
    nc = _get_nc()
    res = run_bass_kernel_spmd(nc, in_maps, core_ids=list(range(NCORES)))
    out = np.empty((B, C, H, W), dtype=np.float32)
    for c in range(NCORES):
        out[c * BLOC:(c + 1) * BLOC] = res.results[c]["out"].reshape(
            BLOC, C, H, W)
    return out


# revision 34
# speedup vs baseline: 1.0289x; 1.0289x over previous
"""Trainium2 Bass kernel for dynamic-depthwise + static conv module.

Computation (per batch b, channel c):
  hid  = leaky_relu(k_v @ W1.T, 0.1)
  kern = (hid @ W2.T).reshape(b*c, 3, 3)        # per-(b,c) dynamic 3x3
  dyn  = leaky_relu(depthwise3x3(x, kern), 0.1)
  res  = conv3x3(x, conv_w) + conv_b
  out  = dyn + res

Sharding: pure data-parallel, B=16 over 8 cores (2 batches/core).

Per-core layout:
  x_pad [128 part = 2 batches x 64 ch, 194*194 zero-padded bf16].  The
  padding is baked into the DRAM tensor host-side, so the load is a
  plain contiguous 128-partition bf16 DMA (no casts, no strided
  descriptors, no on-chip memsets except the stream guards).
  For each spatial tile (2 padded rows, N=388) and each of 9 taps:
  one bf16 matmul per batch, lhsT[k=ci, m] = [conv_w tap | diag(kern)]
  -> PSUM bank per batch: parts 0-63 = static conv, parts 64-127 =
  dynamic.  The two batches run concurrently via PE row tiling (rows
  0-63 / 64-127), which sustains ~2 rows/cycle aggregate.  ACT evicts
  the dynamic half with Prelu (f32); DVE adds bias to the static half;
  a f32 SWDGE accumulate-DMA (dst partitions 0-63 only; larger RMW
  runs than 6 KiB/partition miscompute on HW) folds them; DMA to HBM.
"""
import numpy as np

import concourse.bass as bass
import concourse.tile as tile
import concourse.mybir as mybir

F32 = mybir.dt.float32
F32R = mybir.dt.float32r
BF16 = mybir.dt.bfloat16

B, C, H, W = 16, 64, 192, 192
NCORES = 8
BLOC = B // NCORES          # batches per core
WP = W + 2                  # padded row width
HP = H + 2
PADQ = WP * HP
G = 4                       # guard elems each side of padded buffer
HW = H * W
NTILE = 388                 # 2 padded rows per tile
NT = H // 2                 # 96 tiles

TAPS = [(dy, dx) for dy in (-1, 0, 1) for dx in (-1, 0, 1)]


def _legalize_waits(nc, max_waits=1, evsem_waits=2):
    """This walrus build rejects >1 sync wait on most instructions (2 on
    EventSemaphore). Spill excess waits onto same-engine EventSemaphores
    placed immediately before the instruction."""
    for f in nc.m.functions:
        for bb in f.blocks:
            new_insts = []
            for inst in bb.instructions:
                si = inst.sync_info
                if si is not None and si.on_wait and len(si.on_wait) > max_waits:
                    waits = list(si.on_wait)
                    keep = waits[-max_waits:]
                    spill = waits[:-max_waits]
                    while spill:
                        chunk, spill = spill[:evsem_waits], spill[evsem_waits:]
                        ev = mybir.InstEventSemaphore(
                            name=nc.get_next_instruction_name(),
                            engine=inst.engine,
                            ins=[],
                            outs=[],
                            sync_info=mybir.SyncInfo(on_wait=chunk, on_update=[]),
                        )
                        nc.register_instruction(ev)
                        new_insts.append(ev)
                    inst.sync_info = mybir.SyncInfo(
                        on_wait=keep, on_update=list(si.on_update or [])
                    )
                new_insts.append(inst)
            bb.instructions[:] = new_insts


def _build_nc(loopk=1, xrep=False, hwloop=False):
    nc = bass.Bass()
    xs = nc.dram_tensor("xs", (128, PADQ), BF16, kind="ExternalInput")
    mlpw = nc.dram_tensor("mlpw", (64, BLOC + 64 + 576), F32R,
                          kind="ExternalInput")
    wstat = nc.dram_tensor("wstat", (128, 9 * 128), BF16,
                           kind="ExternalInput")
    ident = nc.dram_tensor("ident", (128, 64), BF16, kind="ExternalInput")
    biasd = nc.dram_tensor("biasd", (128, 1), F32, kind="ExternalInput")
    out = nc.dram_tensor("out", (128, HW), F32, kind="ExternalOutput")

    with tile.TileContext(nc) as tc:
        with (
            tc.tile_pool(name="big", bufs=1) as big,
            tc.tile_pool(name="wpool", bufs=1) as wpool,
            tc.tile_pool(name="work", bufs=3) as work,
        ):
            # ---- persistent tiles ----
            x_pad = big.tile([128, G + PADQ + G], BF16, tag="x_pad")
            wbuf = wpool.tile([128, 9 * 128], BF16, tag="wbuf")
            id_t = wpool.tile([128, 64], BF16, tag="id_t")
            bias_t = wpool.tile([128, 1], F32, tag="bias_t")
            mlpw_t = wpool.tile([64, BLOC + 64 + 576], F32R, tag="mlpw_t")
            kvT_t = mlpw_t[:, 0:BLOC]
            w1t_t = mlpw_t[:, BLOC:BLOC + 64]
            w2t_t = mlpw_t[:, BLOC + 64:BLOC + 64 + 576]
            kern_flat = wpool.tile([BLOC, 576], F32, tag="kern_flat")
            kern128 = wpool.tile([128, 9], F32, tag="kern128")
            hidT = wpool.tile([64, BLOC], F32R, tag="hidT")

            # pre-warm the ACT Prelu table while DMAs run (no input deps)
            warm = wpool.tile([1, 1], F32, tag="warm")
            nc.vector.memset(warm[:], 0.0)
            nc.scalar.activation(warm[:], warm[:],
                                 mybir.ActivationFunctionType.Prelu, alpha=0.1)

            # ---- small constant / weight loads (SP queue, idle here) ----
            nc.sync.dma_start(mlpw_t[:], mlpw[:])
            nc.sync.dma_start(wbuf[:], wstat[:])
            nc.sync.dma_start(id_t[:], ident[:])
            nc.sync.dma_start(bias_t[:], biasd[:])

            # guard zeros (padding itself is baked into the DRAM layout)
            xbv = x_pad.bitcast(mybir.dt.uint16)
            nc.vector.memset(xbv[:, 0:G], 0)
            nc.vector.memset(xbv[:, G + PADQ:G + PADQ + G], 0)

            # ---- MLP: kern = (lrelu(k_v @ W1.T) @ W2.T), f32r ----
            # Scoped PSUM pool: released before pmain so the main loop can
            # use all 8 banks.
            with tc.tile_pool(name="pmlp", bufs=2, space="PSUM") as pmlp:
                p_hid = pmlp.tile([64, 512], F32, tag="pmlp")
                nc.tensor.matmul(p_hid[0:64, 0:BLOC], w1t_t, kvT_t,
                                 start=True, stop=True)
                nc.scalar.activation(hidT[:], p_hid[0:64, 0:BLOC],
                                     mybir.ActivationFunctionType.Prelu,
                                     alpha=0.1)
                p_k1 = pmlp.tile([64, 512], F32, tag="pmlp")
                p_k2 = pmlp.tile([64, 512], F32, tag="pmlp")
                nc.tensor.matmul(p_k1[0:BLOC, 0:288], hidT[:], w2t_t[:, 0:288],
                                 start=True, stop=True)
                nc.tensor.matmul(p_k2[0:BLOC, 0:288], hidT[:],
                                 w2t_t[:, 288:576], start=True, stop=True)
                nc.scalar.copy(kern_flat[:, 0:288], p_k1[0:BLOC, 0:288])
                nc.scalar.copy(kern_flat[:, 288:576], p_k2[0:BLOC, 0:288])
            # reshape (BLOC, 576) -> (128, 9): partition bc = b*64+c
            for b in range(BLOC):
                nc.sync.dma_start(kern128[b * 64:(b + 1) * 64, :],
                                  kern_flat[b:b + 1, :])
            # fill diagonal blocks of wbuf: cols t*128+64 .. t*128+128
            for t in range(9):
                nc.vector.tensor_scalar(
                    wbuf[:, t * 128 + 64:t * 128 + 128], id_t[:],
                    kern128[:, t:t + 1], None, op0=mybir.AluOpType.mult)

            # ---- x loads: contiguous 128-partition bf16 chunks ----
            NCHUNK = 8
            ROWS_PER_CHUNK = (HP + NCHUNK - 1) // NCHUNK   # 25 padded rows

            def load_x_chunk(c):
                if c >= NCHUNK:
                    if not xrep:
                        return
                    c = c % NCHUNK
                r0 = c * ROWS_PER_CHUNK
                r1 = min(HP, r0 + ROWS_PER_CHUNK)
                nc.scalar.dma_start(x_pad[:, G + r0 * WP:G + r1 * WP],
                                    xs[:, r0 * WP:r1 * WP])

            # ---- main loop over 96 spatial tiles (2 padded rows each),
            # grouped by 4 for batched accumulate-DMA + output DMA ----
            GT = 4
            NG = NT // GT
            PREFETCH = 3
            pmain_ctx = tc.tile_pool(name="pmain", bufs=4, space="PSUM")
            pmain = pmain_ctx.__enter__()

            def emit_group(g, do_load, rep=0):
                # 24 groups consume 8 chunks; issue one every 3rd group
                if do_load and g % 3 == 0:
                    load_x_chunk(rep * NCHUNK + g // 3 + PREFETCH)
                dst0 = work.tile([128, GT, 2, W], F32, tag="dst0", bufs=3)
                dst1 = work.tile([128, GT, 2, W], F32, tag="dst1", bufs=3)
                st0 = work.tile([64, GT, 2, W], F32, tag="st0", bufs=3)
                st1 = work.tile([64, GT, 2, W], F32, tag="st1", bufs=3)
                for gi in range(GT):
                    ti = g * GT + gi
                    hp0 = 1 + 2 * ti
                    qs = hp0 * WP
                    pb0f = pmain.tile([128, 512], F32, tag="pb0")
                    pb1f = pmain.tile([128, 512], F32, tag="pb1")
                    pb0 = pb0f[:, 0:NTILE]
                    pb1 = pb1f[:, 0:NTILE]
                    for t, (dy, dx) in enumerate(TAPS):
                        base = G + qs + dy * WP + dx
                        nc.tensor.matmul(
                            pb0[:], wbuf[0:64, t * 128:(t + 1) * 128],
                            x_pad[0:64, base:base + NTILE],
                            start=(t == 0), stop=(t == 8), tile_position=(0, 0))
                        nc.tensor.matmul(
                            pb1[:], wbuf[64:128, t * 128:(t + 1) * 128],
                            x_pad[64:128, base:base + NTILE],
                            start=(t == 0), stop=(t == 8), tile_position=(64, 0))
                    # dyn halves: Prelu -> compact staging (parts 64-127)
                    pv0d = pb0[64:128, :].rearrange("p (r w) -> p r w", w=WP)[:, :, 1:1 + W]
                    pv1d = pb1[64:128, :].rearrange("p (r w) -> p r w", w=WP)[:, :, 1:1 + W]
                    nc.scalar.activation(dst0[64:128, gi, :, :], pv0d,
                                         mybir.ActivationFunctionType.Prelu,
                                         alpha=0.1)
                    nc.scalar.activation(dst1[64:128, gi, :, :], pv1d,
                                         mybir.ActivationFunctionType.Prelu,
                                         alpha=0.1)
                    # static halves: add bias -> compact staging (parts 0-63)
                    pv0 = pb0[0:64, :].rearrange("p (r w) -> p r w", w=WP)[:, :, 1:1 + W]
                    pv1 = pb1[0:64, :].rearrange("p (r w) -> p r w", w=WP)[:, :, 1:1 + W]
                    nc.vector.tensor_scalar(st0[:, gi, :, :], pv0,
                                            bias_t[0:64, :], None,
                                            op0=mybir.AluOpType.add)
                    nc.vector.tensor_scalar(st1[:, gi, :, :], pv1,
                                            bias_t[0:64, :], None,
                                            op0=mybir.AluOpType.add)
                # cross-partition add of leaky(dyn) via SWDGE accumulate DMA
                h0 = 2 * GT * g
                if g < NG - 1:
                    nc.gpsimd.dma_start(st0[:], dst0[64:128, :, :, :],
                                        accum_op=mybir.AluOpType.add)
                    nc.gpsimd.dma_start(st1[:], dst1[64:128, :, :, :],
                                        accum_op=mybir.AluOpType.add)
                    nc.sync.dma_start(out[0:64, h0 * W:(h0 + 2 * GT) * W], st0[:])
                    nc.sync.dma_start(out[64:128, h0 * W:(h0 + 2 * GT) * W],
                                      st1[:])
                else:
                    # finer-grained tail: per-tile accum + out DMAs
                    for gi in range(GT):
                        hh = h0 + 2 * gi
                        nc.gpsimd.dma_start(st0[:, gi, :, :],
                                            dst0[64:128, gi, :, :],
                                            accum_op=mybir.AluOpType.add)
                        nc.gpsimd.dma_start(st1[:, gi, :, :],
                                            dst1[64:128, gi, :, :],
                                            accum_op=mybir.AluOpType.add)
                        nc.sync.dma_start(out[0:64, hh * W:(hh + 2) * W],
                                          st0[:, gi, :, :])
                        nc.sync.dma_start(out[64:128, hh * W:(hh + 2) * W],
                                          st1[:, gi, :, :])

            for c in range(PREFETCH):
                load_x_chunk(c)
            for rep in range(loopk):
                for g in range(NG):
                    emit_group(g, do_load=True, rep=rep)
            pmain_ctx.__exit__(None, None, None)

    _legalize_waits(nc)
    return nc


_NC_CACHE = {}


def _get_nc(loopk=1, xrep=False, hwloop=False):
    key = (loopk, xrep, hwloop)
    if key not in _NC_CACHE:
        _NC_CACHE[key] = _build_nc(loopk, xrep, hwloop)
    return _NC_CACHE[key]


def make_in_maps(x, k_v, W1, W2, conv_w, conv_b):
    import ml_dtypes

    # host-side weight layout prep (parameters only; no input-dependent math)
    blocks = []
    for dy in (0, 1, 2):
        for dx in (0, 1, 2):
            lhsT = conv_w[:, :, dy, dx].T          # [ci, co]
            blocks.append(np.concatenate(
                [lhsT, np.zeros((64, 64), np.float32)], axis=1))
    wstat_half = np.concatenate(blocks, axis=1)     # [64, 9*128]
    wstat = np.tile(wstat_half, (2, 1)).astype(ml_dtypes.bfloat16)
    ident = np.tile(np.eye(64, dtype=np.float32), (2, 1)).astype(
        ml_dtypes.bfloat16)
    biasd = np.tile(conv_b, 2)[:, None].astype(np.float32)
    w1t = W1.T.copy()                               # [64, 64]
    w2t = W2.T.copy()                               # [64, 576]

    # x: zero-padded bf16, baked host-side; partition bc = b*64 + c
    xb = x.astype(ml_dtypes.bfloat16)               # (B, C, H, W)
    in_maps = []
    for cidx in range(NCORES):
        xpad = np.zeros((128, HP, WP), ml_dtypes.bfloat16)
        xpad[:, 1:1 + H, 1:1 + W] = xb[cidx * BLOC:(cidx + 1) * BLOC].reshape(
            128, H, W)
        kvT = k_v[cidx * BLOC:(cidx + 1) * BLOC].T.copy()  # [64, BLOC]
        mlpw = np.concatenate([kvT, w1t, w2t], axis=1)
        in_maps.append({
            "xs": xpad.reshape(128, PADQ), "mlpw": mlpw,
            "wstat": wstat, "ident": ident, "biasd": biasd,
        })
    return in_maps


def expected_core0(full):
    return full[0:BLOC].reshape(128, HW)


def kernel(x, k_v, W1, W2, conv_w, conv_b):
    from concourse.bass_utils import run_bass_kernel_spmd

    x = np.ascontiguousarray(x, dtype=np.float32)
    k_v = np.ascontiguousarray(k_v, dtype=np.float32)
    W1 = np.ascontiguousarray(W1, dtype=np.float32)
    W2 = np.ascontiguousarray(W2, dtype=np.float32)
    conv_w = np.ascontiguousarray(conv_w, dtype=np.float32)
    conv_b = np.ascontiguousarray(conv_b, dtype=np.float32)

    in_maps = make_in_maps(x, k_v, W1, W2, conv_w, conv_b)
    nc = _get_nc()
    res = run_bass_kernel_spmd(nc, in_maps, core_ids=list(range(NCORES)))
    out = np.empty((B, C, H, W), dtype=np.float32)
    for c in range(NCORES):
        out[c * BLOC:(c + 1) * BLOC] = res.results[c]["out"].reshape(
            BLOC, C, H, W)
    return out
